# revision 20
# baseline (speedup 1.0000x reference)
"""Dark-channel loss kernel for Trainium2 (8 NeuronCores, batch-parallel).

reference: loss = mean(|MaxPool3d((3,35,35), stride 1, pad (0,17,17))(1 - img)|)
         = 1 - mean(minpool_{3ch,35x35}(img))        (img in [0,1))

Per-core shard: 4 images [3,512,512] fp32. Pipeline per image (wide ops,
one engine pass per stage; all elementwise work on DVE, staging on ACT,
desc-gen on Pool, transposes/sums on PE):

  1. one SWDGE cast-DMA (split ch01/ch2): img[n] fp32 -> big
     [128, 3*4*512] bf16 (channel k, h-chunk c at cols k*2048+c*512)
  2. DVE: m01 = min(ch0, ch1)  [128,2048]; wt = min(m01, ch2) written into
     a persistent padded tile [128, 4*560] (pads 1.0; data offset 18/block)
  3. DVE: W-direction van Herk sliding min-35: segmented fwd/bwd min-scans
     (masks add +BIG at block starts) over [128,2240]
  4. DVE: mw = min(r[.,b,1:513], s[.,b,35:547]) -> [128,2048]
  5. PE: 16 transposes (128x128 bf16) -> PSUM [128,512] x4
  6. ACT: copy each PSUM tile into padded H tile [128, 4*560] (data off 18)
  7. DVE: H-direction scans over [128,2240] + combine -> mhw [128,2048]
  8. PE: ones-matmul accumulates partition sums of mhw into PSUM cs [1,512]
Final: DVE reduce cs -> [1,1]; host: loss = 1 - sum(partials)/(N*H*W).

bf16 is safe: min commutes with monotone rounding, so the computed minima
are exactly bf16-rounded true minima; |loss error| <= 2^-9 * mean(min) ~ 5e-7.

Asymmetric van Herk pad: with data offset d >= 17 in each 560 block
(16 blocks of 35), window start for output j is a=j+d-17; combine is
min(bwd[a], fwd[a+34]). d=18 keeps PSUM/transpose offsets 4B-aligned.
"""

import os
import numpy as np

N_CORES = 8
N, C, H, W = 32, 3, 512, 512
PER = N // N_CORES          # images per core
P = 128
WIN = 35
NB = 16                     # blocks per 560-segment
SEG = NB * WIN              # 560
D = 18                      # data offset within a segment (4B aligned)
CH = 4                      # h-chunks / w-chunks per image
LW = CH * SEG               # 2240: 4 segments side by side
BIG = 1e9

TIMELOOP = int(os.environ.get("DC_TIMELOOP", "0"))

_cached_nc = None


def _build_nc(timeloop=TIMELOOP):
    import concourse.bacc as bacc
    import concourse.mybir as mybir
    from concourse.tile import TileContext
    from concourse.masks import make_identity

    dt = mybir.dt
    Alu = mybir.AluOpType

    nc = bacc.Bacc("TRN2")
    img = nc.declare_dram_parameter("img", [PER, C, H, W], dt.float32,
                                    isOutput=False)
    out = nc.declare_dram_parameter("out", [1, 1], dt.float32, isOutput=True)

    with TileContext(nc) as tc:
        with (
            tc.tile_pool(name="consts", bufs=1) as consts,
            tc.tile_pool(name="big", bufs=2) as bigp,
            tc.tile_pool(name="mm", bufs=2) as mmp,
            tc.tile_pool(name="sr", bufs=2) as srp,
            tc.tile_pool(name="psT", bufs=4, space="PSUM") as psT,
            tc.tile_pool(name="psS", bufs=1, space="PSUM") as psS,
        ):
            # --- constants (emitted before the loop; inside the loop the
            # first DMA descriptor-gens go ahead of nothing on Pool, and the
            # masks/pads are set on DVE, which is idle while DMA 0 lands) ---
            mask_f = consts.tile([P, LW], dt.bfloat16, tag="mask_f")
            mask_b = consts.tile([P, LW], dt.bfloat16, tag="mask_b")
            ident = consts.tile([P, P], dt.bfloat16, tag="ident")
            ones = consts.tile([P, 1], dt.bfloat16, tag="ones")
            wring = [consts.tile([P, LW], dt.bfloat16, tag=f"wt{i}",
                                 name=f"wt{i}") for i in range(2)]
            uring = [consts.tile([P, LW], dt.bfloat16, tag=f"ut{i}",
                                 name=f"ut{i}") for i in range(2)]
            cs = psS.tile([1, 512], dt.float32, tag="csum")

            def emit_consts():
                nc.vector.memset(mask_f[:], 0.0)
                nc.vector.memset(mask_b[:], 0.0)
                mf3 = mask_f[:].rearrange("p (nb w) -> p nb w", w=WIN)
                mb3 = mask_b[:].rearrange("p (nb w) -> p nb w", w=WIN)
                nc.vector.memset(mf3[:, :, 0:1], BIG)
                nc.vector.memset(mb3[:, :, WIN - 1:WIN], BIG)
                make_identity(nc, ident[:])
                nc.gpsimd.memset(ones[:], 1.0)
                for t in wring + uring:
                    t3 = t[:].rearrange("p (nb l) -> p nb l", l=SEG)
                    nc.vector.memset(t3[:, :, 0:D], 1.0)
                    nc.vector.memset(t3[:, :, D + 512:SEG], 1.0)

            emit_consts()

            import contextlib
            loop_ctx = (tc.For_i(0, timeloop, 1) if timeloop
                        else contextlib.nullcontext())
            with loop_ctx:
                _body_emit(nc, tc, mybir, Alu, dt, img, bigp, mmp, srp,
                           psT, mask_f, mask_b, ident, ones, wring, uring, cs)

            # final sum of cs on ACT (accumulate register), freeing DVE
            sink = consts.tile([1, 512], dt.float32, tag="sink")
            tot = consts.tile([1, 1], dt.float32, tag="tot")
            nc.scalar.activation(out=sink[:], in_=cs[:],
                                 func=mybir.ActivationFunctionType.Copy,
                                 accum_out=tot[:])
            nc.sync.dma_start(out=out[:], in_=tot[:])

    nc.compile()
    return nc


def _body_emit(nc, tc, mybir, Alu, dt, img, bigp, mmp, srp, psT,
               mask_f, mask_b, ident, ones, wring, uring, cs):
    n_mm = PER * CH
    mm_i = 0

    def scans(src, srcname, n):
        """fwd+bwd segmented min-scans over a padded [P, LW] tile."""
        s = srp.tile([P, LW], dt.bfloat16, tag=f"s_{srcname}",
                     name=f"s_{srcname}_{n}")
        r = srp.tile([P, LW], dt.bfloat16, tag=f"r_{srcname}",
                     name=f"r_{srcname}_{n}")
        nc.vector.tensor_tensor_scan(
            out=s[:], data0=mask_f[:], data1=src[:], initial=BIG,
            op0=Alu.add, op1=Alu.min)
        nc.vector.tensor_tensor_scan(
            out=r[:, ::-1], data0=mask_b[:, ::-1], data1=src[:, ::-1],
            initial=BIG, op0=Alu.add, op1=Alu.min)
        return s, r

    def combine(s, r, dst):
        """dst[p, b*512+j] = min(r[p,b,j+D-17], s[p,b,j+D+17]) for 4 segs."""
        s3 = s[:].rearrange("p (nb l) -> p nb l", l=SEG)
        r3 = r[:].rearrange("p (nb l) -> p nb l", l=SEG)
        d3 = dst[:].rearrange("p (nb l) -> p nb l", l=512)
        nc.vector.tensor_tensor(
            out=d3[:, :, :], in0=r3[:, :, D - 17:D - 17 + 512],
            in1=s3[:, :, D + 17:D + 17 + 512], op=Alu.min)

    def load(n):
        # SWDGE cast-DMA, one mega-load per image (fp32->bf16 in the DMA);
        # measured best in-kernel despite lower loads-only throughput than
        # HWDGE fp32 (the ACT-cast variant serialized the pipeline).
        # split per image: (ch0,ch1) then ch2, so m01 starts 1/3 sooner
        big = bigp.tile([P, 3 * 2048], dt.bfloat16, tag="big",
                        name=f"big_{n}")
        b4 = big[:].rearrange("p (k c w) -> p k c w", k=3, c=CH)
        i4 = img[n].rearrange("k (c p) w -> p k c w", p=P)
        nc.gpsimd.dma_start(out=b4[:, 0:2], in_=i4[:, 0:2])
        nc.gpsimd.dma_start(out=b4[:, 2:3], in_=i4[:, 2:3])
        return big

    def w_phase(n, big):
        # channel min (2 wide TTs)
        m01 = mmp.tile([P, 2048], dt.bfloat16, tag="m01", name=f"m01_{n}")
        nc.vector.tensor_tensor(out=m01[:], in0=big[:, 0:2048],
                                in1=big[:, 2048:4096], op=Alu.min)
        wt = wring[n % 2]
        wt3 = wt[:].rearrange("p (nb l) -> p nb l", l=SEG)
        m3 = m01[:].rearrange("p (nb l) -> p nb l", l=512)
        b3 = big[:, 4096:6144].rearrange("p (nb l) -> p nb l", l=512)
        nc.vector.tensor_tensor(out=wt3[:, :, D:D + 512], in0=m3[:, :, :],
                                in1=b3[:, :, :], op=Alu.min)

        # W-direction sliding min
        sW, rW = scans(wt, "w", n)
        mw = mmp.tile([P, 2048], dt.bfloat16, tag="mw", name=f"mw_{n}")
        combine(sW, rW, mw)

        # transpose [512,512] -> 4 PSUM tiles; ACT stages into padded ut
        ut = uring[n % 2]
        ut3 = ut[:].rearrange("p (nb l) -> p nb l", l=SEG)
        for j in range(CH):
            pt = psT.tile([P, 512], dt.bfloat16, tag="pt",
                          name=f"pt_{n}_{j}")
            for c in range(CH):
                nc.tensor.transpose(pt[:, P * c:P * (c + 1)],
                                    mw[:, 512 * c + P * j:512 * c + P * (j + 1)],
                                    ident[:])
            nc.scalar.copy(out=ut3[:, j, D:D + 512], in_=pt[:])
        return ut

    def h_phase(n, ut, per_chunk=False):
        nonlocal mm_i
        if not per_chunk:
            # H-direction sliding min, all 4 w-chunks in two scans
            sH, rH = scans(ut, "u", n)
            mhw = mmp.tile([P, 2048], dt.bfloat16, tag="mhw",
                           name=f"mhw_{n}")
            combine(sH, rH, mhw)
            for j in range(CH):
                nc.tensor.matmul(cs[:], ones[:],
                                 mhw[:, 512 * j:512 * (j + 1)],
                                 start=(mm_i == 0), stop=(mm_i == n_mm - 1),
                                 skip_group_check=True)
                mm_i += 1
            return
        # last image: per-chunk scans start as soon as each ACT copy lands
        ut3 = ut[:].rearrange("p (nb l) -> p nb l", l=SEG)
        for j in range(CH):
            s = srp.tile([P, SEG], dt.bfloat16, tag="s_uc",
                         name=f"s_uc_{n}_{j}")
            r = srp.tile([P, SEG], dt.bfloat16, tag="r_uc",
                         name=f"r_uc_{n}_{j}")
            nc.vector.tensor_tensor_scan(
                out=s[:], data0=mask_f[:, 0:SEG], data1=ut3[:, j, :],
                initial=BIG, op0=Alu.add, op1=Alu.min)
            nc.vector.tensor_tensor_scan(
                out=r[:, ::-1], data0=mask_b[:, 0:SEG][:, ::-1],
                data1=ut3[:, j, ::-1], initial=BIG,
                op0=Alu.add, op1=Alu.min)
            mhw = mmp.tile([P, 512], dt.bfloat16, tag="mhw_c",
                           name=f"mhw_c_{n}_{j}")
            nc.vector.tensor_tensor(
                out=mhw[:], in0=r[:, D - 17:D - 17 + 512],
                in1=s[:, D + 17:D + 17 + 512], op=Alu.min)
            nc.tensor.matmul(cs[:], ones[:], mhw[:],
                             start=(mm_i == 0), stop=(mm_i == n_mm - 1),
                             skip_group_check=True)
            mm_i += 1

    # software pipeline: H-phase of image n-1 is emitted after the W-phase
    # of image n, so DVE never stalls on transposes/staging of its own image
    bigs, uts = {}, {}
    bigs[0] = load(0)
    for n in range(PER):
        if n + 1 < PER:
            bigs[n + 1] = load(n + 1)
        uts[n] = w_phase(n, bigs.pop(n))
        if n - 1 in uts:
            h_phase(n - 1, uts.pop(n - 1))
    h_phase(PER - 1, uts.pop(PER - 1), per_chunk=True)


def _get_nc():
    global _cached_nc
    if _cached_nc is None:
        _cached_nc = _build_nc()
    return _cached_nc


def _finish(results):
    partials = np.array([float(results[i]["out"][0, 0])
                         for i in range(N_CORES)])
    loss = 1.0 - float(np.sum(partials, dtype=np.float64)) / (N * H * W)
    return np.asarray(loss, dtype=np.float32)


def kernel(img):
    from concourse.bass_utils import run_bass_kernel_spmd
    img_np = np.asarray(img, dtype=np.float32)
    assert img_np.shape == (N, C, H, W), img_np.shape
    shards = img_np.reshape(N_CORES, PER, C, H, W)
    in_maps = [{"img": np.ascontiguousarray(shards[i])}
               for i in range(N_CORES)]
    res = run_bass_kernel_spmd(_get_nc(), in_maps, list(range(N_CORES)))
    return _finish(res.results)
